# revision 1
# baseline (speedup 1.0000x reference)
"""Per-sample 21x21 blur (grouped conv, reflect pad) on trn2, 8 NeuronCores.

Problem: input [16, 3, 768, 768] f32, kernel [16, 21, 21] f32 (one blur
kernel per sample, shared across channels), reflect-pad 10, output
[16, 3, 768, 768] f32.

Strategy (data-parallel over batch, 2 samples/core):
  The conv is mapped to TensorE matmuls via a Toeplitz factorization over
  image rows.  For an output row-block of M=96 rows, the 116 input rows
  covering it are contracted against a banded [116, 96] matrix T_dx holding
  kernel column dx on its diagonals; the 21 dx terms accumulate in PSUM with
  the rhs shifted along the free (column) axis by dx:

    out[yb*96+m, x0+n] = sum_dx  T_dx[r, m] * pad[yb*96+r, x0+dx+n]

  Operands are cast to float32r (fp32 with 11-bit mantissa) so the PE
  streams 1 column/cycle instead of fp32's 1/4 rate.
"""
import sys

sys.path.insert(0, "/opt/trn_rl_repo")

import numpy as np

N_CORES = 8
B, C, H, W = 16, 3, 768, 768
KS = 21          # kernel size
PAD = 10         # reflect pad
HP = H + 2 * PAD  # 788
WP = W + 2 * PAD  # 788
MBLK = 96        # output rows per matmul block
KBLK = MBLK + KS - 1  # 116 input rows per block
NBLK = 384       # output cols per PSUM chunk (2 chunks of 384 = 768)
YBLKS = H // MBLK  # 8
SPC = B // N_CORES  # samples per core = 2
IMGS = SPC * C      # images per core = 6

_prog_cache = {}


def build_program(reps=1):
    import concourse.bacc as bacc
    import concourse.mybir as mybir
    from concourse.tile import TileContext

    nc = bacc.Bacc(None, target_bir_lowering=False)
    x = nc.declare_dram_parameter("x", [IMGS, HP, WP], mybir.dt.float32, isOutput=False)
    w = nc.declare_dram_parameter(
        "w", [KBLK, SPC * KS, MBLK], mybir.dt.float32, isOutput=False
    )
    y = nc.declare_dram_parameter("y", [IMGS, H, W], mybir.dt.float32, isOutput=True)

    with TileContext(nc) as tc:
        with (
            tc.tile_pool(name="wpool", bufs=1) as wpool,
            tc.tile_pool(name="xpool", bufs=3) as xpool,
            tc.tile_pool(name="xrpool", bufs=3) as xrpool,
            tc.tile_pool(name="opool", bufs=3) as opool,
            tc.tile_pool(name="psum", bufs=4, space="PSUM") as psum_pool,
        ):
            w_f32 = wpool.tile([KBLK, SPC * KS, MBLK], mybir.dt.float32)
            nc.sync.dma_start(out=w_f32[:, :, :], in_=w[:, :, :])
            w_r = wpool.tile([KBLK, SPC * KS, MBLK], mybir.dt.float32r)
            nc.vector.tensor_copy(out=w_r[:, :, :], in_=w_f32[:, :, :])

            for _ in range(reps):
                for img in range(IMGS):
                    s = img // C
                    for yb in range(YBLKS):
                        x_f32 = xpool.tile([KBLK, WP], mybir.dt.float32)
                        nc.sync.dma_start(
                            out=x_f32[:, :],
                            in_=x[img, yb * MBLK : yb * MBLK + KBLK, :],
                        )
                        x_r = xrpool.tile([KBLK, WP], mybir.dt.float32r)
                        nc.vector.tensor_copy(out=x_r[:, :], in_=x_f32[:, :])

                        out_sb = opool.tile([MBLK, W], mybir.dt.float32)
                        for ci, x0 in enumerate(range(0, W, NBLK)):
                            ps = psum_pool.tile([MBLK, NBLK], mybir.dt.float32)
                            for dx in range(KS):
                                nc.tensor.matmul(
                                    ps[:, :],
                                    w_r[:, s * KS + dx, :],
                                    x_r[:, x0 + dx : x0 + dx + NBLK],
                                    start=(dx == 0),
                                    stop=(dx == KS - 1),
                                )
                            nc.vector.tensor_copy(
                                out=out_sb[:, x0 : x0 + NBLK], in_=ps[:, :]
                            )
                        nc.sync.dma_start(
                            out=y[img, yb * MBLK : (yb + 1) * MBLK, :],
                            in_=out_sb[:, :],
                        )
    nc.compile()
    return nc


def _toeplitz_weights(kern_pair):
    """kern_pair: [SPC, 21, 21] -> [KBLK, SPC*21, MBLK] f32."""
    wt = np.zeros((KBLK, SPC * KS, MBLK), np.float32)
    for s in range(SPC):
        for dx in range(KS):
            col = kern_pair[s, :, dx]  # taps over dy
            for m in range(MBLK):
                wt[m : m + KS, s * KS + dx, m] = col
    return wt


def make_in_maps(inp, kern):
    pad = np.pad(inp, ((0, 0), (0, 0), (PAD, PAD), (PAD, PAD)), mode="reflect")
    in_maps = []
    for c in range(N_CORES):
        s0 = c * SPC
        x_core = pad[s0 : s0 + SPC].reshape(IMGS, HP, WP)
        w_core = _toeplitz_weights(kern[s0 : s0 + SPC])
        in_maps.append({"x": np.ascontiguousarray(x_core), "w": w_core})
    return in_maps


def kernel(input, kernel):
    from concourse.bass_utils import run_bass_kernel_spmd

    inp = np.asarray(input, dtype=np.float32)
    kern = np.asarray(kernel, dtype=np.float32)
    in_maps = make_in_maps(inp, kern)

    if "nc" not in _prog_cache:
        _prog_cache["nc"] = build_program()
    nc = _prog_cache["nc"]

    res = run_bass_kernel_spmd(nc, in_maps, list(range(N_CORES)))
    out = np.empty((B, C, H, W), np.float32)
    for c in range(N_CORES):
        out[c * SPC : (c + 1) * SPC] = res.results[c]["y"].reshape(SPC, C, H, W)
    return out
